# revision 73
# baseline (speedup 1.0000x reference)
"""DenseCRF mean-field kernel for Trainium2 (8 NeuronCores, data parallel).

Math per sample (B=8 samples -> 1 per core):
    Q0 = softmax(unary, axis=class)
    repeat 5x:  Q <- softmax(Q - compat @ ((pos_w+bi_w) * gauss7(Q)), axis=class)
(`image` is unused by the reference math.)

The 7x7 gaussian blur is separable with reflect padding, expressed as two
banded matrix multiplies on the TensorEngine:
    pass1: T1[w, h'] = sum_h Q[h, w] * AT[h, h']              (H-conv, transposed out)
    pass2: L[h',w'] = Q[h',w'] + sum_w T1[w, h'] * (-s*AT)[w, w']  (W-conv + identity)
where AT = A.T, A the [n,n] reflect conv matrix of g, s = pos_w + bi_w
(times compat diagonal).

Engine balance (all four compute engines + DMA kept busy):
  - PE: pass1 uses disjoint start=True column ownership (no full-width
    PSUM-init matmul); pass2 = identity matmul + band windows.
  - Act: all exps, 1/3 of the pass1 PSUM->SBUF copies, and the f16 cast
    of the softmax reciprocal (GpSimd has no PSUM port, so copies can
    only go to Act/DVE).
  - DVE: 2/3 of the copies, the class-sum adds for 3 groups, the
    reciprocal, and the early normalize-muls.
  - Pool/GpSimd: one class-sum group and the late normalize-muls
    (SBUF-only ops; reads the f32 reciprocal directly - Pool cost is
    dtype-independent).
Critical-path care: the per-round softmax tail (s -> recip -> mul c0) is
split in h-block quarters (NHALF=4) so the next round's pass1 is released
a few us after the last exp; the remaining DVE muls are emitted
interleaved with the next round's pass1 so the in-order DVE queue never
starves PE of PSUM-copy results; group partials fold with the Pool group
last so DVE never waits on Pool mid-round. (Measured dead ends on this
cost model + backend: gpsimd scalar_tensor_tensor and gpsimd PSUM reads
fail walrus codegen; fp8 DoubleRow shifts work onto the busier DVE; DMA
cannot touch PSUM; deeper pass2 lag or Pool-routed copies lengthen the
round's critical path.)

Per core the sample stays resident in SBUF as fp16 between iterations;
HBM traffic is the initial unary load (f32, DMA-saturated behind a
4-deep staging pipeline) and the final Q store (f16 halves streamed per
class, upcast on host).
"""

from contextlib import ExitStack

import numpy as np

import concourse.bacc as bacc
import concourse.tile as tile
from concourse import mybir
from concourse.bass_utils import run_bass_kernel_spmd

F32 = mybir.dt.float32
F16 = mybir.dt.float16

B, C, H, W = 8, 21, 512, 512
KSIZE, SIGMA = 7, 2.0
NUM_ITERATIONS = 5
PB = 128                       # partition block
R = KSIZE // 2                 # band half-width (3)
BANDW = PB + 2 * R             # max band window width (134)

# engine schedules (identity-compat fast path).
# Strategy: Pool (gpsimd) carries most PSUM->SBUF copies (its TensorCopy is
# charged at the 0.60 default efficiency, 1.52us/half) so the DVE is free for
# the muls/adds it does at 2x f16 (1.13us/class); Act does exps + a few
# copies; a small Pool add-group absorbs residual DVE overflow.
POOL_ADD_GROUP = 5             # classes 0..4 accumulate on Pool
DVE_GROUPS = ((5, 10), (10, 15), (15, 19))  # three DVE class-sum groups
# classes c-2, c-1 (19, 20) are folded directly into the final f32 sum by
# the TensorEngine (idle at the round boundary), keeping the DVE chain short
POOL_ADD_GROUPS = frozenset((0,))  # class-sum groups on Pool
POOL_MUL_CLASSES = frozenset(range(10, 20))  # Pool normalize-mul classes
FIN_POOL_MULS = frozenset()    # final-round muls on Pool
NHALF = 4                      # tail-chain split factor
POOL_COPY_PAIR1 = False        # Pool as pure copier (needs LAG=2 slack)
COPY_ACT_CLASSES = 1           # class 0's copies both on Act
_CRR_N, _CRR_D = 1, 3          # 1 of every 3 PAIR0 copies goes to Act


def copy_eng(cc, mp, rr):
    """Engine for the (class, PAIR-group) PSUM->SBUF copy. Pool is a pure
    copier (all PAIR1 + two PAIR0s); Act takes the even classes' PAIR0;
    DVE the odd ones. With the lag-2 pass2 pipeline each copy has ~2 class
    periods of slack, so Pool's latency/jitter is tolerable."""
    if not POOL_COPY_PAIR1:
        if cc < COPY_ACT_CLASSES:
            return "act"
        on_act = (rr[0] % _CRR_D) < _CRR_N
        rr[0] += 1
        return "act" if on_act else "dve"
    if mp == 0:
        if cc % 2 == 0 and cc < 20:
            return "act"
        return "pool" if cc in (1, 11) else "dve"
    return "pool"


def _gauss1d():
    coords = np.arange(KSIZE, dtype=np.float64) - KSIZE // 2
    g = np.exp(-(coords ** 2) / (2.0 * SIGMA ** 2))
    return g / g.sum()


def _conv_matrix(n, g):
    r = len(g) // 2
    A = np.zeros((n, n), np.float64)
    for i in range(n):
        for t in range(len(g)):
            j = i + t - r
            if j < 0:
                j = -j
            if j >= n:
                j = 2 * n - 2 - j
            A[i, j] += g[t]
    return A  # filt = A @ x  (reflect boundary)


def _windows(n):
    return [(max(0, PB * i - R), min(n, PB * i + PB + R)) for i in range(n // PB)]


def build_program(c=C, hb=H // PB, w=W, iters=NUM_ITERATIONS, n_cores=8,
                  b2_per_class=False, offdiag=None):
    """Build the per-core Bass program.

    offdiag: None for (scaled-)identity compat, else the full [c,c] compat
    matrix -> generic (slow) class-mix path with DRAM-resident E.
    """
    h = hb * PB
    wb = w // PB
    wins_h = _windows(h)
    wins_w = _windows(w)
    n_b2 = c if b2_per_class else 1
    generic = offdiag is not None

    nc = bacc.Bacc("TRN2", target_bir_lowering=False, debug=False,
                   num_devices=n_cores)
    U = nc.dram_tensor("unary", [c, h, w], F32, kind="ExternalInput")
    BD1 = nc.dram_tensor("band1", [hb, PB, h], F16, kind="ExternalInput")
    BD2 = nc.dram_tensor("band2", [n_b2, wb, PB, BANDW], F16, kind="ExternalInput")
    IDN = nc.dram_tensor("ident", [PB, PB], F16, kind="ExternalInput")
    OUT = nc.dram_tensor("out", [c, h, w], F16 if not generic else F32,
                         kind="ExternalOutput")
    EDR = nc.dram_tensor("escr", [c, h, w], F16) if generic else None

    if generic:
        n_grp = 3 if c >= 6 else 1
        grps = np.array_split(np.arange(c), n_grp)
        grp_of, first_in_grp = {}, {}
        for gi, g in enumerate(grps):
            for k, ccls in enumerate(g):
                grp_of[int(ccls)] = gi
                first_in_grp[int(ccls)] = (k == 0)
    else:
        # group 0 (classes 0..POOL_ADD_GROUP-1) accumulates on Pool; groups
        # 1..3 on DVE; the final class is folded straight into the f32 sum.
        # Second member of each group does the pair-init add.
        grp_of, idx_in_grp = {}, {}
        bounds = [(0, POOL_ADD_GROUP)] + list(DVE_GROUPS)
        for gi, (lo, hi) in enumerate(bounds):
            for k, ccls in enumerate(range(lo, hi)):
                grp_of[ccls] = gi
                idx_in_grp[ccls] = k
        n_grp = len(bounds)
        last_cls = c - 1
        grp_close = {gi: hi - 1 for gi, (lo, hi) in enumerate(bounds)}

    with tile.TileContext(nc) as tc, ExitStack() as ctx:
        singles = ctx.enter_context(tc.tile_pool(name="singles", bufs=1))
        t1ps_pool = ctx.enter_context(tc.tile_pool(name="t1ps", bufs=2, space="PSUM"))
        lps_pool = ctx.enter_context(tc.tile_pool(name="lps", bufs=2, space="PSUM"))
        t1sb_pool = ctx.enter_context(tc.tile_pool(name="t1sb", bufs=3))
        stage_pool = ctx.enter_context(tc.tile_pool(name="stage", bufs=2))
        spart_pool = ctx.enter_context(tc.tile_pool(name="spart", bufs=1))
        sums_pool = ctx.enter_context(tc.tile_pool(name="sums", bufs=2))
        mix_pool = ctx.enter_context(tc.tile_pool(name="mix", bufs=2))
        out_pool = ctx.enter_context(tc.tile_pool(name="fout", bufs=5))

        # ---- persistent / constant SBUF ----
        qres = singles.tile([PB, c, hb, w], F16, tag="qres")
        b1 = singles.tile([PB, hb, h], F16, tag="b1")
        b2 = singles.tile([PB, n_b2, wb, BANDW], F16, tag="b2")
        ident = singles.tile([PB, PB], F16, tag="ident")

        def load_consts():
            for i in range(hb):
                nc.sync.dma_start(out=b1[:, i, :], in_=BD1[i])
            for j in range(n_b2):
                for i in range(wb):
                    nc.sync.dma_start(out=b2[:, j, i, :], in_=BD2[j, i])
            nc.sync.dma_start(out=ident[:], in_=IDN[:])

        if generic:
            load_consts()

        spart = {}
        copy_rr = [0]  # round-robin state for pass1 copy engine
        rh_ref = [None]  # rh tile of the most recent finish_round
        r_ref = [None]   # r (f32) tile of the most recent finish_round

        # ------------------- identity-path accumulation -------------------
        # incremental combine: tpre accumulates closed groups off the
        # critical path; the final class's E joins in the f32 sum directly.
        tpre = {}

        def pool_add(out, in0, in1):
            nc.gpsimd.tensor_add(out=out, in0=in0, in1=in1)

        def pool_mul(out, in0, in1):
            nc.gpsimd.tensor_mul(out=out, in0=in0, in1=in1)

        def accum_E_class(cc):
            """Accumulate E (== qres[:, cc]) into its group partial."""
            if cc >= last_cls - 1:
                return  # folded into the final f32 sum by the PE directly
            gi = grp_of[cc]
            add = pool_add if gi in POOL_ADD_GROUPS else (
                lambda out, in0, in1: nc.vector.tensor_add(
                    out=out, in0=in0, in1=in1))
            k = idx_in_grp[cc]
            if k == 0:
                pass  # wait for pair-init with class lo+1
            elif k == 1:
                t = spart_pool.tile([PB, hb, w], F16, tag=f"sp_{gi}")
                spart[gi] = t
                add(t[:], qres[:, cc - 1], qres[:, cc])
            else:
                add(spart[gi][:], spart[gi][:], qres[:, cc])
            # fold closed groups as they become ready; the Pool group (g0)
            # joins LAST so the in-order DVE queue never waits on Pool
            # (Pool's adds run late, behind its normalize-mul backlog).
            if cc == grp_close[2]:
                t = spart_pool.tile([PB, hb, w], F16, tag="t01")
                tpre[0] = t
                nc.vector.tensor_add(out=t[:], in0=spart[1][:], in1=spart[2][:])
            elif cc == grp_close[3]:
                nc.vector.tensor_add(out=tpre[0][:], in0=tpre[0][:],
                                     in1=spart[3][:])
                nc.vector.tensor_add(out=tpre[0][:], in0=tpre[0][:],
                                     in1=spart[0][:])

        # ------------------- generic-path accumulation --------------------
        def accum_E(cc, m2, e_ap):
            gi = grp_of[cc]
            if first_in_grp[cc] and (gi, m2) not in spart:
                t = sums_pool.tile([PB, w], F16, tag=f"spm_{gi}_{m2}")
                spart[(gi, m2)] = t
                nc.vector.tensor_copy(out=t[:], in_=e_ap)
            else:
                t = spart[(gi, m2)]
                nc.vector.tensor_add(out=t[:], in0=t[:], in1=e_ap)

        def emit_exp_generic(cc, m2, src_ap):
            est = stage_pool.tile([PB, w], F16, tag="est")
            nc.scalar.activation(out=est[:], in_=src_ap,
                                 func=mybir.ActivationFunctionType.Exp)
            accum_E(cc, m2, est[:])
            nc.sync.dma_start(out=EDR[cc, m2 * PB:(m2 + 1) * PB, :],
                              in_=est[:])

        def finish_round(last):
            """Emit the softmax tail. Returns DVE muls deferred for
            interleaving with the next round's pass1 (non-last rounds)."""
            if not generic:
                # tail chain (split along the h-block axis):
                # s = tpre + E[last_cls] (f32) -> recip (DVE). The first
                # normalize-muls read r (f32) directly so they do not wait
                # for rh; rh (f16, for the remaining DVE muls) is produced
                # on the idle Act engine.
                hh = hb // NHALF
                r = sums_pool.tile([PB, hb, w], F32, tag="r", bufs=1)
                rh = sums_pool.tile([PB, hb, w], F16, tag="rh")
                rh_ref[0] = rh
                r_ref[0] = r

                def emit_half_sum(u):
                    # s = tpre + E[c-2] + E[c-1] on the (idle) TensorEngine
                    # via identity matmuls accumulating in PSUM
                    sps = lps_pool.tile([PB, hh, w], F32, tag="lps")
                    for j in range(hh):
                        m2 = u * hh + j
                        nc.tensor.matmul(sps[:, j, :], ident[:],
                                         tpre[0][:, m2, :],
                                         start=True, stop=False)
                        nc.tensor.matmul(sps[:, j, :], ident[:],
                                         qres[:, last_cls - 1, m2, :],
                                         start=False, stop=False)
                        nc.tensor.matmul(sps[:, j, :], ident[:],
                                         qres[:, last_cls, m2, :],
                                         start=False, stop=True)
                    return sps

                if not last:
                    # depth-first halves: s(PE) -> recip -> mul(c0) per half,
                    # so the next round's pass1(c0) starts on half 0 early;
                    # rh halves on Act so mul(c1) is not rh-gated either
                    for u in range(NHALF):
                        sl = slice(u * hh, (u + 1) * hh)
                        sps = emit_half_sum(u)
                        nc.vector.reciprocal_approx_fast(out=r[:, sl],
                                                         in_=sps[:])
                        nc.scalar.copy(out=rh[:, sl], in_=r[:, sl])
                        nc.vector.tensor_mul(out=qres[:, 0, sl],
                                             in0=qres[:, 0, sl], in1=rh[:, sl])
                    for u in range(NHALF):
                        sl = slice(u * hh, (u + 1) * hh)
                        nc.vector.tensor_mul(out=qres[:, 1, sl],
                                             in0=qres[:, 1, sl], in1=rh[:, sl])
                    dve_pending = []
                    for cc in range(2, c):
                        if cc in POOL_MUL_CLASSES:
                            # Pool cost is dtype-independent: read r directly
                            pool_mul(qres[:, cc], qres[:, cc], r[:])
                        else:
                            dve_pending.append(cc)
                    return dve_pending, []
                else:
                    # final round: keep everything on DVE and stream the
                    # f16 result out per half as soon as it is ready
                    for u in range(NHALF):
                        sl = slice(u * hh, (u + 1) * hh)
                        sps = emit_half_sum(u)
                        nc.vector.reciprocal_approx_fast(out=r[:, sl],
                                                         in_=sps[:])
                    tpre.clear()
                    # cast rh per h-half so the first output wave (and its
                    # DMA stream) is released after two recip quarters
                    hh2c = hb // 2
                    nc.scalar.copy(out=rh[:, 0:hh2c], in_=r[:, 0:hh2c])
                    nc.scalar.copy(out=rh[:, hh2c:], in_=r[:, hh2c:])
                    for cc in range(c):
                        fo = out_pool.tile([PB, hb, w], F16, tag="fout")
                        hh2 = hb // 2
                        for u in range(2):
                            sl = slice(u * hh2, (u + 1) * hh2)
                            # first classes read r (f32) so the output DMA
                            # stream starts before rh lands
                            nrm = r if cc < 1 else rh
                            nc.vector.tensor_mul(out=fo[:, sl],
                                                 in0=qres[:, cc, sl],
                                                 in1=nrm[:, sl])
                            # dest rows are (m2*PB + p); (p, m2, x) order
                            nc.sync.dma_start(
                                out=OUT[cc].rearrange(
                                    "(m p) w -> p m w", p=PB)[:, sl],
                                in_=fo[:, sl])
                    return [], []
            else:
                rh = []
                for m2 in range(hb):
                    s = sums_pool.tile([PB, w], F32, tag=f"sm_{m2}")
                    if n_grp == 1:
                        nc.vector.tensor_copy(out=s[:], in_=spart[(0, m2)][:])
                    else:
                        nc.vector.tensor_add(out=s[:], in0=spart[(0, m2)][:],
                                             in1=spart[(1, m2)][:])
                        for gi in range(2, n_grp):
                            nc.vector.tensor_add(out=s[:], in0=s[:],
                                                 in1=spart[(gi, m2)][:])
                    r = sums_pool.tile([PB, w], F32, tag=f"rm_{m2}")
                    nc.vector.reciprocal_approx_fast(out=r[:], in_=s[:])
                    rhm = sums_pool.tile([PB, w], F16, tag=f"rhm_{m2}")
                    nc.vector.tensor_copy(out=rhm[:], in_=r[:])
                    rh.append(rhm)
                for cc in range(c):
                    for m2 in range(hb):
                        esrc = stage_pool.tile([PB, w], F16, tag="eld")
                        nc.sync.dma_start(
                            out=esrc[:], in_=EDR[cc, m2 * PB:(m2 + 1) * PB, :])
                        if not last:
                            nc.vector.tensor_mul(out=qres[:, cc, m2, :],
                                                 in0=esrc[:], in1=rh[m2][:])
                        else:
                            fo = stage_pool.tile([PB, w], F32, tag="fom")
                            nc.vector.tensor_mul(out=fo[:], in0=esrc[:],
                                                 in1=rh[m2][:])
                            nc.sync.dma_start(
                                out=OUT[cc, m2 * PB:(m2 + 1) * PB, :], in_=fo[:])
            spart.clear()
            return [], []

        # ---- init: Q0 = softmax(unary) ----
        if not generic:
            hh0 = hb // 2
            for cc in range(c):
                st = stage_pool.tile([PB, hb, w], F32, tag="uin", bufs=4)
                # load + exp in h-halves: the final class's first half
                # releases the softmax tail chain ~3us earlier
                usrc = U[cc].rearrange("(m p) w -> p m w", p=PB)
                for u in range(2):
                    sl = slice(u * hh0, (u + 1) * hh0)
                    nc.sync.dma_start(out=st[:, sl], in_=usrc[:, sl])
                    nc.scalar.activation(
                        out=qres[:, cc, sl], in_=st[:, sl],
                        func=mybir.ActivationFunctionType.Exp)
                accum_E_class(cc)
            load_consts()  # band matrices are not needed until pass1
        else:  # generic
            for cc in range(c):
                for m2 in range(hb):
                    st = stage_pool.tile([PB, w], F32, tag="uin")
                    nc.sync.dma_start(out=st[:],
                                      in_=U[cc, m2 * PB:(m2 + 1) * PB, :])
                    emit_exp_generic(cc, m2, st[:])
        dve_pend, pool_pend = finish_round(last=False)

        PAIR = 2 if (hb % 2 == 0 and wb % 2 == 0 and not generic) else 1

        def emit_pass1(cc, src_fn):
            t1sb = t1sb_pool.tile([PB, wb, h], F16, tag="t1sb")
            for mp in range(0, wb, PAIR):
                t1ps = t1ps_pool.tile([PB, PAIR, h], F32, tag="t1ps")
                for ml in range(PAIR):
                    m = mp + ml
                    # disjoint start=True ownership: block 0 owns [0, PB+R);
                    # block i>0 accumulates its R-left-overlap [PB*i-R, PB*i+R)
                    # then owns [PB*i+R, hi_i) with a fresh start=True.
                    nc.tensor.matmul(
                        t1ps[:, ml, 0:PB + R],
                        src_fn(0, slice(m * PB, (m + 1) * PB)),
                        b1[:, 0, 0:PB + R],
                        start=True, stop=(hb == 1))
                    for i in range(1, hb):
                        lo, hi = wins_h[i]
                        mid = PB * i + R
                        lhsT = src_fn(i, slice(m * PB, (m + 1) * PB))
                        nc.tensor.matmul(
                            t1ps[:, ml, lo:mid], lhsT, b1[:, i, lo:mid],
                            start=False, stop=False)
                        nc.tensor.matmul(
                            t1ps[:, ml, mid:hi], lhsT, b1[:, i, mid:hi],
                            start=True, stop=(i == hb - 1))
                # PSUM->SBUF copy routing by (class, pair): Act / Pool / DVE.
                eng = "act" if generic else copy_eng(cc, mp, copy_rr)
                if eng == "act":
                    nc.scalar.copy(out=t1sb[:, mp:mp + PAIR, :], in_=t1ps[:])
                elif eng == "pool":
                    nc.gpsimd.tensor_copy(out=t1sb[:, mp:mp + PAIR, :],
                                          in_=t1ps[:])
                else:
                    nc.vector.tensor_copy(out=t1sb[:, mp:mp + PAIR, :],
                                          in_=t1ps[:])
            return t1sb

        def emit_pass2(cc, t1sb, last):
            b2c = b2[:, cc if n_b2 > 1 else 0]
            for m2p in range(0, hb, PAIR):
                lps = lps_pool.tile([PB, PAIR, w], F32, tag="lps")
                for ml in range(PAIR):
                    m2 = m2p + ml
                    nc.tensor.matmul(lps[:, ml, 0:w], ident[:],
                                     qres[:, cc, m2, :],
                                     start=True, stop=False)
                    for i2 in range(wb):
                        lo, hi = wins_w[i2]
                        nc.tensor.matmul(
                            lps[:, ml, lo:hi],
                            t1sb[:, i2, m2 * PB:(m2 + 1) * PB],
                            b2c[:, i2, 0:hi - lo],
                            start=False, stop=(i2 == wb - 1))
                if not generic:
                    nc.scalar.activation(
                        out=qres[:, cc, m2p:m2p + PAIR, :], in_=lps[:],
                        func=mybir.ActivationFunctionType.Exp)
                else:
                    for ml in range(PAIR):
                        emit_exp_generic(cc, m2p + ml, lps[:, ml, :])
            if not generic:
                accum_E_class(cc)

        # ---- iterations (class loop software-pipelined LAG deep) ----
        LAG = 1
        for k in range(iters):
            last = (k == iters - 1)
            prevq = []
            for cc in range(c):
                if generic:
                    msrc = mix_pool.tile([PB, hb, w], F16, tag="mix")
                    nz = [j for j in range(c) if offdiag[cc, j] != 0.0]
                    for i in range(hb):
                        if not nz:
                            nc.vector.memset(msrc[:, i, :], 0.0)
                        else:
                            j0 = nz[0]
                            nc.vector.tensor_scalar_mul(
                                out=msrc[:, i, :], in0=qres[:, j0, i, :],
                                scalar1=float(offdiag[cc, j0]))
                            for j in nz[1:]:
                                nc.vector.scalar_tensor_tensor(
                                    out=msrc[:, i, :], in0=qres[:, j, i, :],
                                    scalar=float(offdiag[cc, j]),
                                    in1=msrc[:, i, :],
                                    op0=mybir.AluOpType.mult,
                                    op1=mybir.AluOpType.add)

                    def src_fn(i, mcols, _m=msrc):
                        return _m[:, i, mcols]
                else:
                    def src_fn(i, mcols, _c=cc):
                        return qres[:, _c, i, mcols]

                t1sb = emit_pass1(cc, src_fn)
                if not generic and dve_pend:
                    nc.vector.tensor_mul(out=qres[:, dve_pend[0]],
                                         in0=qres[:, dve_pend[0]], in1=rh_ref[0][:])
                    dve_pend.pop(0)
                while pool_pend and pool_pend[0] <= cc + 3:
                    kk = pool_pend.pop(0)
                    pool_mul(qres[:, kk], qres[:, kk], r_ref[0][:])
                prevq.append((cc, t1sb))
                if len(prevq) >= LAG + 1:
                    pc, pt = prevq.pop(0)
                    emit_pass2(pc, pt, last)
            for pc, pt in prevq:
                emit_pass2(pc, pt, last)
            dve_pend, pool_pend = finish_round(last=last)

    nc.compile()
    return nc


def _prep_consts(c, h, w, scale, compat):
    g = _gauss1d()
    AT_h = _conv_matrix(h, g).T
    AT_w = _conv_matrix(w, g).T
    band1 = np.zeros((h // PB, PB, h), np.float16)
    for i in range(h // PB):
        band1[i] = AT_h[i * PB:(i + 1) * PB, :].astype(np.float16)

    diag = np.diag(compat).astype(np.float64)
    is_diag = bool(np.count_nonzero(compat - np.diag(diag)) == 0)
    uniform = is_diag and bool(np.all(diag == diag[0]))

    offdiag = None
    if is_diag:
        n_b2 = 1 if uniform else c
        scales = [float(scale) * float(diag[0])] if uniform else \
                 [float(scale) * float(d) for d in diag]
    else:
        n_b2 = 1
        scales = [float(scale)]
        offdiag = compat.astype(np.float64)

    band2 = np.zeros((n_b2, w // PB, PB, BANDW), np.float16)
    for j in range(n_b2):
        for i, (lo, hi) in enumerate(_windows(w)):
            band2[j, i, :, 0:hi - lo] = (
                -scales[j] * AT_w[i * PB:(i + 1) * PB, lo:hi]).astype(np.float16)
    ident = np.eye(PB, dtype=np.float16)
    return band1, band2, ident, (n_b2 > 1), offdiag


_prog_cache = {}


def kernel(unary, image, pos_w, bi_w, compatibility):
    unary = np.asarray(unary, dtype=np.float32)
    compat = np.asarray(compatibility, dtype=np.float32)
    scale = float(np.asarray(pos_w)) + float(np.asarray(bi_w))
    b, c, h, w = unary.shape
    assert (b, c, h, w) == (B, C, H, W), (b, c, h, w)

    band1, band2, ident, per_class, offdiag = _prep_consts(c, h, w, scale, compat)
    key = (scale, compat.tobytes())
    if key not in _prog_cache:
        _prog_cache[key] = build_program(
            c=c, hb=h // PB, w=w, iters=NUM_ITERATIONS, n_cores=B,
            b2_per_class=per_class, offdiag=offdiag)
    nc = _prog_cache[key]

    in_maps = [{"unary": unary[i], "band1": band1, "band2": band2,
                "ident": ident} for i in range(B)]
    res = run_bass_kernel_spmd(nc, in_maps, list(range(B)))
    out = np.stack([res.results[i]["out"] for i in range(B)], axis=0)
    return out.astype(np.float32)


if __name__ == "__main__":
    rng = np.random.default_rng(0)
    u = rng.standard_normal((B, C, H, W), dtype=np.float32)
    img = rng.random((B, 3, H, W), dtype=np.float32)
    o = kernel(u, img, np.float32(3.0), np.float32(10.0),
               np.eye(C, dtype=np.float32))
    print(o.shape, o.dtype, float(o.sum()))

